# revision 10
# baseline (speedup 1.0000x reference)
"""Trainium2 Bass kernel for nn_ContrastiveLoss (B=2048, D=4096, C=1000, 8 cores).

loss = CE(y_preds, y_true) + pos + neg, with
  pos = mean over same-label pairs i<j of (1 - cos(x_i, x_j))
  neg = mean over the 16 pairs (0,j), j=1..16 of relu(cos(x_0, x_j))

Math refactor (exact up to fp rounding): with xn_i = x_i / max(|x_i|, eps),
  sum_{i<j, y_i=y_j} cos_ij = (||G||_F^2 - sum_i |xn_i|^2) / 2,
  where G[c] = sum_{i: y_i=c} xn_i  (per-class sums).
So no BxB similarity matrix is needed. Rows are bucketed onto cores by label
range (125 classes/core), each core computes G for its classes via a one-hot
matmul, plus its shard of the CE rows; the neg part needs only a 17x17 Gram.
Per-core partial sums are combined on the host (~20 scalar flops).

Schedule: the G matmul accumulates tile-by-tile (t-outer) into four D-quarter
PSUM regions kept live across the whole row loop (8 banks), so the PE overlaps
with the DMA/normalize pipeline instead of running as a cold tail. Class-count
and 17x17-Gram matmuls run first (PSUM banks free before G claims all 8).
Norm reductions and scaling alternate between ACT and DVE to balance engines.
"""

import numpy as np

import concourse.bacc as bacc
import concourse.tile as tile
from concourse import mybir
from concourse import bass_utils

F32 = mybir.dt.float32
BF16 = mybir.dt.bfloat16
I32 = mybir.dt.int32
ALU = mybir.AluOpType
ACTF = mybir.ActivationFunctionType
AX = mybir.AxisListType

B, D, C = 2048, 4096, 1000
NCORES = 8
CLS_PER = C // NCORES          # 125 classes per core
NCLS = 128                     # one-hot width (padded)
RB_MAIN = 384                  # padded bucket rows (buckets are ~256 +/- 15)
RB_SAFE = 512                  # fallback if a bucket overflows 384
CE_ROWS = B // NCORES          # 256
CE_T = CE_ROWS // 128          # 2
KNEG = 17                      # rows 0..16 for the negative pairs
KD = D // 128                  # 32 contraction chunks for the neg Gram
NQ = 4                         # D quarters for the G psum
QW = D // NQ                   # 1024
EPS2 = 1e-16                   # eps^2 for max(norm, 1e-8)

# out vector layout: [ce0, ce1, g0, g1, g2, g3, m2, negsum]
OUTW = 8


def build_nc(rb=RB_MAIN):
    nt = rb // 128
    nc = bacc.Bacc("TRN2", target_bir_lowering=False)

    xb_d = nc.dram_tensor("xb", [nt, 128, D], F32, kind="ExternalInput")
    yb_d = nc.dram_tensor("yb", [nt, 128, 1], I32, kind="ExternalInput")
    yp_d = nc.dram_tensor("yp", [CE_T, 128, C], F32, kind="ExternalInput")
    yt_d = nc.dram_tensor("yt", [CE_T, 128, 1], I32, kind="ExternalInput")
    xng_d = nc.dram_tensor("xng", [KD, 128, KNEG], F32, kind="ExternalInput")
    out_d = nc.dram_tensor("out", [1, OUTW], F32, kind="ExternalOutput")

    with tile.TileContext(nc) as tc:
        with (
            tc.tile_pool(name="singles", bufs=1) as singles,
            tc.tile_pool(name="xpool", bufs=3) as xpool,
            tc.tile_pool(name="xnpool", bufs=nt) as xnpool,
            tc.tile_pool(name="apool", bufs=nt) as apool,
            tc.tile_pool(name="sqpool", bufs=2) as sqpool,
            tc.tile_pool(name="cepool", bufs=2) as cepool,
            tc.tile_pool(name="small", bufs=4) as small,
            tc.tile_pool(name="psg", bufs=1, space="PSUM") as psg,
            tc.tile_pool(name="pss", bufs=1, space="PSUM") as pss,
        ):
            # ---- constants ----
            iota_cls = singles.tile([128, NCLS], F32)
            nc.gpsimd.iota(iota_cls[:], pattern=[[1, NCLS]], base=0,
                           channel_multiplier=0,
                           allow_small_or_imprecise_dtypes=True)
            iota_ce = singles.tile([128, C], F32)
            nc.gpsimd.iota(iota_ce[:], pattern=[[1, C]], base=0,
                           channel_multiplier=0,
                           allow_small_or_imprecise_dtypes=True)
            ones_f = singles.tile([128, 1], F32)
            nc.vector.memset(ones_f[:], 1.0)
            ones_b = singles.tile([128, 1], BF16)
            nc.vector.memset(ones_b[:], 1.0)

            V = singles.tile([128, 6], F32)
            nc.vector.memset(V[:], 0.0)
            out_sb = singles.tile([1, OUTW], F32)
            nc.vector.memset(out_sb[:], 0.0)

            # ---- one-hot label tiles (tiny DMAs; ready before xb lands) ----
            a_tiles = []
            for t in range(nt):
                ybt = small.tile([128, 1], I32, tag="ybt")
                nc.sync.dma_start(out=ybt[:], in_=yb_d[t])
                ybf = small.tile([128, 1], F32, tag="ybf")
                nc.vector.tensor_copy(out=ybf[:], in_=ybt[:])
                at = apool.tile([128, NCLS], BF16, tag="a")
                nc.vector.tensor_scalar(out=at[:], in0=iota_cls[:],
                                        scalar1=ybf[:], scalar2=None,
                                        op0=ALU.is_equal)
                a_tiles.append(at)

            # class counts m = ones^T @ A -> [1, NCLS]; m2 = sum(m^2)
            mpsum = pss.tile([1, NCLS], F32, tag="mpsum")
            for t in range(nt):
                nc.tensor.matmul(mpsum[:], ones_b[:], a_tiles[t][:],
                                 start=(t == 0), stop=(t == nt - 1))
            msq = small.tile([1, NCLS], F32, tag="msq")
            nc.scalar.activation(out=msq[:], in_=mpsum[:], func=ACTF.Square,
                                 accum_out=out_sb[:, 6:7])

            # ---- negative pairs: 17x17 Gram in K-layout (early PE work) ----
            xng = singles.tile([128, KD, KNEG], F32)
            nc.sync.dma_start(out=xng[:],
                              in_=xng_d[:].rearrange("k p j -> p k j"))
            g17 = pss.tile([KNEG, KNEG], F32, tag="g17")
            for k in range(KD):
                nc.tensor.matmul(g17[:], xng[:, k, :], xng[:, k, :],
                                 start=(k == 0), stop=(k == KD - 1))
            sqn = singles.tile([128, KD, KNEG], F32)
            nc.vector.tensor_mul(sqn[:], xng[:], xng[:])
            sqk = singles.tile([128, KNEG], F32)
            nc.vector.reduce_sum(out=sqk[:],
                                 in_=sqn[:].rearrange("p k j -> p j k"),
                                 axis=AX.X)
            n2row = pss.tile([1, KNEG], F32, tag="n2row")
            nc.tensor.matmul(n2row[:], ones_f[:], sqk[:], start=True,
                             stop=True)
            nn17 = small.tile([1, KNEG], F32, tag="nn17")
            nc.vector.tensor_scalar_max(nn17[:], n2row[:], EPS2)
            nc.scalar.sqrt(out=nn17[:], in_=nn17[:])
            inv17 = small.tile([1, KNEG], F32, tag="inv17")
            nc.vector.reciprocal(out=inv17[:], in_=nn17[:])
            srow = small.tile([1, KNEG], F32, tag="srow")
            nc.vector.tensor_mul(srow[:], g17[0:1, :], inv17[:])
            nc.vector.tensor_scalar_mul(srow[:], srow[:], inv17[:, 0:1])
            nc.vector.tensor_scalar_max(srow[:], srow[:], 0.0)
            nc.vector.reduce_sum(out=out_sb[:, 7:8], in_=srow[0:1, 1:KNEG],
                                 axis=AX.X)

            # ---- pos: stream row tiles, normalize, accumulate G --------
            # Two sequential D-halves of 2048 (4 PSUM banks each, same slot):
            # half 0 accumulates t-outer while tiles stream; half 1 reruns the
            # (SBUF-resident) xn tiles as a short warm burst afterwards.
            HW2 = D // 2
            inv_all = singles.tile([128, nt], F32)
            n2_all = singles.tile([128, nt], F32)
            xn_tiles = []
            for t in range(nt):
                n2c = n2_all[:, t : t + 1]
                invc = inv_all[:, t : t + 1]
                xt = xpool.tile([128, D], F32, tag="xt")
                nc.sync.dma_start(out=xt[:], in_=xb_d[t])
                sq = sqpool.tile([128, D], F32, tag="sq")
                if t % 2 == 0:  # ACT: n2 via Square+accumulate
                    nc.scalar.activation(out=sq[:], in_=xt[:],
                                         func=ACTF.Square, accum_out=n2c)
                else:           # DVE: n2 via (x+0)*x with accumulate
                    nc.vector.scalar_tensor_tensor(
                        out=sq[:], in0=xt[:], scalar=0.0, in1=xt[:],
                        op0=ALU.add, op1=ALU.mult, accum_out=n2c)
                # inv = 1 / max(sqrt(n2), 1e-8)
                nc.vector.tensor_scalar_max(n2c, n2c, EPS2)
                nc.scalar.sqrt(out=n2c, in_=n2c)
                nc.vector.reciprocal(out=invc, in_=n2c)
                # xn = x * inv (bf16), alternating engine
                xnt = xnpool.tile([128, D], BF16, tag="xn")
                if t % 2 == 0:
                    nc.vector.tensor_scalar_mul(xnt[:], xt[:], invc)
                else:
                    nc.scalar.activation(out=xnt[:], in_=xt[:],
                                         func=ACTF.Copy, scale=invc)
                xn_tiles.append(xnt)
            for h in range(2):
                gh = psg.tile([128, HW2], F32, name=f"gh{h}", tag="gh")
                for t in range(nt):
                    for s in range(HW2 // 512):
                        lo = h * HW2 + s * 512
                        nc.tensor.matmul(
                            gh[:, s * 512 : (s + 1) * 512],
                            a_tiles[t][:], xn_tiles[t][:, lo : lo + 512],
                            start=(t == 0), stop=(t == nt - 1),
                        )
                gsq = sqpool.tile([128, HW2], F32, tag="gsq")
                nc.scalar.activation(out=gsq[:], in_=gh[:],
                                     func=ACTF.Square,
                                     accum_out=V[:, 2 + h : 3 + h])

            # ---- cross entropy shard (fills the G-matmul window) ----
            for i in range(CE_T):
                zt = cepool.tile([128, C], F32, tag="zt")
                nc.sync.dma_start(out=zt[:], in_=yp_d[i])
                ytt = small.tile([128, 1], I32, tag="ytt")
                nc.sync.dma_start(out=ytt[:], in_=yt_d[i])
                ytf = small.tile([128, 1], F32, tag="ytf")
                nc.vector.tensor_copy(out=ytf[:], in_=ytt[:])
                mx = small.tile([128, 1], F32, tag="mx")
                nc.vector.reduce_max(out=mx[:], in_=zt[:], axis=AX.X)
                negm = small.tile([128, 1], F32, tag="negm")
                nc.vector.tensor_scalar_mul(negm[:], mx[:], -1.0)
                et = cepool.tile([128, C], F32, tag="et")
                se = small.tile([128, 1], F32, tag="se")
                nc.scalar.activation(out=et[:], in_=zt[:], func=ACTF.Exp,
                                     bias=negm[:], scale=1.0, accum_out=se[:])
                ls = small.tile([128, 1], F32, tag="ls")
                nc.scalar.activation(out=ls[:], in_=se[:], func=ACTF.Ln)
                # zy = z[row, y[row]] via fused (iota==y)*z with accumulate
                prod = cepool.tile([128, C], F32, tag="prod")
                zy = small.tile([128, 1], F32, tag="zy")
                nc.vector.scalar_tensor_tensor(
                    out=prod[:], in0=iota_ce[:], scalar=ytf[:], in1=zt[:],
                    op0=ALU.is_equal, op1=ALU.mult, accum_out=zy[:])
                # ce = (mx + ls) - zy
                t1 = small.tile([128, 1], F32, tag="t1")
                nc.vector.tensor_add(t1[:], mx[:], ls[:])
                nc.vector.tensor_sub(V[:, i : i + 1], t1[:], zy[:])

            # ---- partition-reduce V via ones matmul, assemble output ----
            red = pss.tile([1, 6], F32, tag="red")
            nc.tensor.matmul(red[:], ones_f[:], V[:], start=True, stop=True)
            nc.vector.tensor_copy(out=out_sb[:, 0:6], in_=red[:])
            nc.sync.dma_start(out=out_d[:], in_=out_sb[:])

    nc.finalize()
    return nc


_NC_CACHE = {}


def _get_nc(rb):
    if rb not in _NC_CACHE:
        _NC_CACHE[rb] = build_nc(rb)
    return _NC_CACHE[rb]


def make_in_maps(xs, y_preds, y_true, rb):
    nt = rb // 128
    xs = np.ascontiguousarray(np.asarray(xs, dtype=np.float32))
    yp = np.ascontiguousarray(np.asarray(y_preds, dtype=np.float32))
    y = np.asarray(y_true).astype(np.int32).ravel()
    assert xs.shape == (B, D) and yp.shape == (B, C) and y.shape == (B,)

    xng = np.ascontiguousarray(xs[:KNEG].T).reshape(KD, 128, KNEG)
    in_maps = []
    for k in range(NCORES):
        sel = np.nonzero((y >= k * CLS_PER) & (y < (k + 1) * CLS_PER))[0]
        nk = len(sel)
        assert nk <= rb, f"bucket {k} overflow: {nk} > {rb}"
        xb = np.zeros((rb, D), dtype=np.float32)
        xb[:nk] = xs[sel]
        yb = np.full((rb, 1), -1, dtype=np.int32)
        yb[:nk, 0] = y[sel] - k * CLS_PER
        in_maps.append({
            "xb": xb.reshape(nt, 128, D),
            "yb": yb.reshape(nt, 128, 1),
            "yp": yp[k * CE_ROWS : (k + 1) * CE_ROWS].reshape(CE_T, 128, C),
            "yt": y[k * CE_ROWS : (k + 1) * CE_ROWS]
                 .astype(np.int32).reshape(CE_T, 128, 1),
            "xng": xng,
        })
    return in_maps


def combine(outs):
    """outs: [NCORES][1, OUTW] partial vectors -> final loss scalar."""
    o = np.stack([np.asarray(x, dtype=np.float64).ravel() for x in outs])
    ce_sum = o[:, 0].sum() + o[:, 1].sum()
    g2 = o[:, 2:6].sum()
    m2 = o[:, 6].sum()
    neg = o[0, 7]
    loss_ce = ce_sum / B
    cnt = (m2 - B) / 2.0
    sum_s = (g2 - B) / 2.0
    pos_sum = cnt - sum_s
    loss_pos = pos_sum / max(cnt, 1.0) if cnt > 0 else 0.0
    loss_neg = neg / (KNEG - 1)
    return np.array(loss_ce + loss_pos + loss_neg, dtype=np.float32)


def kernel(xs, y_preds, y_true, _trace=False):
    y = np.asarray(y_true).astype(np.int32).ravel()
    max_bucket = max(
        int(((y >= k * CLS_PER) & (y < (k + 1) * CLS_PER)).sum())
        for k in range(NCORES))
    rb = RB_MAIN if max_bucket <= RB_MAIN else RB_SAFE
    nc = _get_nc(rb)
    in_maps = make_in_maps(xs, y_preds, y_true, rb)
    res = bass_utils.run_bass_kernel_spmd(
        nc, in_maps, core_ids=list(range(NCORES)), trace=_trace,
    )
    loss = combine([r["out"] for r in res.results])
    if _trace:
        return loss, res
    return loss
